# revision 23
# baseline (speedup 1.0000x reference)
"""Multi-head attention (B=2, S=2048, D=1024, H=16, causal) on 8 trn2 cores.

Sharding: core c -> batch b = c//4, head group g = c%4 (4 heads of 64 dims):
data parallel over batch, tensor/head parallel within it (W_q/W_k/W_v split
column-wise, W_o row-wise per head group).  Each core computes Q/K/V
projections for its head group over the full sequence, causal flash-style
attention, and the partial output projection A_g @ Wo.T[g_rows, :].  The host
pre-transposes activations/weight slices to fp16, sums the 4 bf16 output
partials per batch (the row-parallel unshard), and adds bo.

v2 schedule (vs the phase-ordered baseline):
  - DMA loads stream in compute priority order (wq/wk + x chunk-0 columns
    first, per-e granularity for the first chunk so the first projection
    chain starts ~3us in), V/x data for later chunks trickle behind.
  - V-store chains, next-chunk Q/K chains and previous-chunk output bursts
    are a single filler queue drained one item per attention tile, so the
    PE never waits on the ACT exp stream.
  - exp is issued per head plane ([128,w] x2 instead of [128,2,w]) so the
    AV matmul of plane 0 starts while plane 1 is still exping.
  - softmax normalization reads PSUM directly (reciprocal + multiply), no
    den/A-copy staging; causal strip muls run on gpsimd.
  - output partials are bf16, copied into [128,2,1024] staging tiles and
    DMA'd two s-tiles at a time.
  - last chunk: pair-0 out-proj contributions (d=0 matmuls) run as fillers
    inside pair-1's attention stream into bf16 partials; the tail only runs
    the 8 d=1 matmuls + add-combine + DMA.
"""

import collections

import ml_dtypes
import numpy as np

import concourse.bacc as bacc
import concourse.mybir as mybir
import concourse.tile as tile
from concourse.bass_utils import run_bass_kernel_spmd

F32 = mybir.dt.float32
BF16 = mybir.dt.float16  # fp16: same PE speed as bf16, 4x the mantissa
F8 = mybir.dt.float8e4
NP_BF16 = np.float16
NP_F8 = ml_dtypes.float8_e4m3
DR = mybir.MatmulPerfMode.DoubleRow

S = 2048        # sequence length
E = 1024        # model dim (contraction for projections)
DG = 256        # head-group dim (4 heads x 64)
DH = 64         # head dim
NH = 4          # heads per core
ET = E // 128   # 8 e-tiles
ST = S // 128   # 16 s-tiles
SC = 512        # sequence chunk (psum free dim)
NSC = S // SC   # 4 chunks
SCALE = 1.0 / np.sqrt(DH)

_CACHED = {}


def _build():
    nc = bacc.Bacc("TRN2", target_bir_lowering=False, debug=False, num_devices=8)

    xqT = nc.dram_tensor("xqT", [E, S], BF16, kind="ExternalInput")
    xkT = nc.dram_tensor("xkT", [E, S], BF16, kind="ExternalInput")
    xvT = nc.dram_tensor("xvT", [E, S], BF16, kind="ExternalInput")
    wqT = nc.dram_tensor("wqT", [E, DG], BF16, kind="ExternalInput")
    wkT = nc.dram_tensor("wkT", [E, DG], BF16, kind="ExternalInput")
    wvT = nc.dram_tensor("wvT", [E, DG], BF16, kind="ExternalInput")
    woT = nc.dram_tensor("woT", [DG, E], BF16, kind="ExternalInput")
    bq = nc.dram_tensor("bq", [DG], F32, kind="ExternalInput")
    bk = nc.dram_tensor("bk", [DG], F32, kind="ExternalInput")
    bv = nc.dram_tensor("bv", [DG], F32, kind="ExternalInput")
    out = nc.dram_tensor("out", [S, E], BF16, kind="ExternalOutput")

    with tile.TileContext(nc) as tc:
        with (
            tc.tile_pool(name="persist", bufs=1) as pp,
            tc.tile_pool(name="xin", bufs=1) as xin,
            tc.tile_pool(name="epool", bufs=6) as epool,
            tc.tile_pool(name="opool", bufs=2) as opool,
            tc.tile_pool(name="small", bufs=2) as small,
            tc.tile_pool(name="ps_a", bufs=2, space="PSUM") as ps_a,
            tc.tile_pool(name="ps_e", bufs=2, space="PSUM") as ps_e,
            tc.tile_pool(name="ps_o", bufs=1, space="PSUM") as ps_o,
        ):
            # ---- persistent tiles ----
            wq_sb = pp.tile([128, ET, DG], BF16, tag="wq")
            wk_sb = pp.tile([128, ET, DG], BF16, tag="wk")
            wv_sb = pp.tile([128, ET, DG], BF16, tag="wv")
            wo_sb = pp.tile([128, 2, E], BF16, tag="wo")
            bq_sb = pp.tile([128, 2], F32, tag="bq")
            bk_sb = pp.tile([128, 2], F32, tag="bk")
            bv_sb = pp.tile([1, DG], F32, tag="bv")
            bvb = pp.tile([128, DG], F32, tag="bvb")

            xq_sb = xin.tile([128, ET, S], BF16, tag="xq", name="xq")
            xk_sb = xin.tile([128, ET, S], BF16, tag="xk", name="xk")
            xv_sb = xin.tile([128, ET, S], BF16, tag="xv", name="xv")

            # ---- DMA stream, priority ordered ----
            # band 0: projection weights + chunk-0 activations (per-e so the
            # first chain starts as soon as its first slices land)
            nc.gpsimd.dma_start(bq_sb[:], bq.ap().rearrange("(a p) -> p a", p=128))
            nc.gpsimd.dma_start(bk_sb[:], bk.ap().rearrange("(a p) -> p a", p=128))
            nc.gpsimd.dma_start(bv_sb[:], bv.ap().rearrange("(o d) -> o d", o=1))
            wq_view = wqT.ap().rearrange("(a p) d -> p a d", p=128)
            nc.gpsimd.dma_start(wq_sb[:, 0:ET // 2, :], wq_view[:, 0:ET // 2, :])
            nc.gpsimd.dma_start(wq_sb[:, ET // 2:, :], wq_view[:, ET // 2:, :])
            nc.gpsimd.dma_start(wk_sb[:], wkT.ap().rearrange("(a p) d -> p a d", p=128))
            for e in range(ET):
                nc.sync.dma_start(xq_sb[:, e, 0:SC],
                                  xqT.ap()[128 * e:128 * (e + 1), 0:SC])
                nc.scalar.dma_start(xk_sb[:, e, 0:SC],
                                    xkT.ap()[128 * e:128 * (e + 1), 0:SC])
            nc.gpsimd.dma_start(wv_sb[:], wvT.ap().rearrange("(a p) d -> p a d", p=128))
            # band 1: V chunk 0, then later x chunks in need order
            xv_view = xvT.ap().rearrange("(a p) s -> p a s", p=128)
            xq_view = xqT.ap().rearrange("(a p) s -> p a s", p=128)
            xk_view = xkT.ap().rearrange("(a p) s -> p a s", p=128)
            nc.sync.dma_start(xv_sb[:, :, 0:SC], xv_view[:, :, 0:SC])
            nc.scalar.dma_start(xq_sb[:, :, SC:2 * SC], xq_view[:, :, SC:2 * SC])
            nc.sync.dma_start(xk_sb[:, :, SC:2 * SC], xk_view[:, :, SC:2 * SC])
            nc.gpsimd.dma_start(wo_sb[:], woT.ap().rearrange("(a p) j -> p a j", p=128))
            nc.scalar.dma_start(xv_sb[:, :, SC:2 * SC], xv_view[:, :, SC:2 * SC])
            nc.sync.dma_start(xv_sb[:, :, 2 * SC:3 * SC], xv_view[:, :, 2 * SC:3 * SC])
            nc.scalar.dma_start(xq_sb[:, :, 2 * SC:4 * SC], xq_view[:, :, 2 * SC:4 * SC])
            nc.sync.dma_start(xk_sb[:, :, 2 * SC:4 * SC], xk_view[:, :, 2 * SC:4 * SC])
            nc.scalar.dma_start(xv_sb[:, :, 3 * SC:4 * SC], xv_view[:, :, 3 * SC:4 * SC])

            # bias row broadcast for the V-store add (gpsimd, once)
            nc.gpsimd.partition_broadcast(bvb[:], bv_sb[:1, :])

            # causal strip: strip[p, f] = 1.0 if f - p >= 384 else 0.0
            strip = pp.tile([128, 896], BF16, tag="strip")
            nc.gpsimd.memset(strip[:], 1.0)
            nc.gpsimd.affine_select(
                out=strip[:],
                in_=strip[:],
                compare_op=mybir.AluOpType.is_ge,
                fill=0.0,
                base=-384,
                pattern=[[1, 896]],
                channel_multiplier=-1,
            )

            # ---- compute helpers ----
            qt_sb = [pp.tile([128, S], BF16, tag=f"qt{d}", name=f"qt{d}") for d in range(2)]
            kt_sb = [pp.tile([128, S], BF16, tag=f"kt{d}", name=f"kt{d}") for d in range(2)]
            at_sb = [pp.tile([128, S], BF16, tag=f"at{d}", name=f"at{d}") for d in range(2)]
            vst = [pp.tile([128, NH * (DH + 1)], BF16, tag=f"vst{st}", name=f"vst{st}")
                   for st in range(ST)]
            for st in range(ST):
                nc.gpsimd.memset(vst[st][:], 1.0)

            def proj_chain(x_sb, w_sb, b_sb, dst, sc, d):
                ps = ps_a.tile([128, SC], F32, tag="ps_proj",
                               name=f"pj{dst[0].name}{sc}{d}")
                for e in range(ET):
                    nc.tensor.matmul(
                        ps[:],
                        w_sb[:, e, 128 * d:128 * (d + 1)],
                        x_sb[:, e, SC * sc:SC * (sc + 1)],
                        start=(e == 0),
                        stop=(e == ET - 1),
                    )
                nc.vector.tensor_scalar_add(
                    dst[d][:, SC * sc:SC * (sc + 1)], ps[:], b_sb[:, d:d + 1]
                )

            def q_chain(sc, d):
                proj_chain(xq_sb, wq_sb, bq_sb, qt_sb, sc, d)

            def k_chain(sc, d):
                proj_chain(xk_sb, wk_sb, bk_sb, kt_sb, sc, d)

            def vst_chain(st):
                ps = ps_a.tile([128, SC], F32, tag="ps_proj", name=f"psv{st}")
                col = 128 * st
                for e in range(ET):
                    nc.tensor.matmul(
                        ps[:, 0:DG],
                        xv_sb[:, e, col:col + 128],
                        wv_sb[:, e, :],
                        start=(e == 0),
                        stop=(e == ET - 1),
                    )
                # psum + broadcast bias -> v columns of the store (ones cols stay)
                nc.vector.tensor_add(
                    vst[st][:].rearrange("p (h x) -> p h x", h=NH)[:, :, 0:DH],
                    ps[:, 0:DG].rearrange("p (h x) -> p h x", h=NH),
                    bvb[:].rearrange("p (h x) -> p h x", h=NH),
                )

            def score_tile(pair, qc, t):
                """Both heads' score matmuls -> one 2-bank psum; per-plane exp."""
                qt, kt = qt_sb[pair], kt_sb[pair]
                diag = t >= 4 * qc
                dd = 128 * t - SC * qc if diag else 0
                w = SC - dd
                pse = ps_e.tile([128, 2, SC], F32, tag="pse", name=f"pse{pair}{qc}{t}")
                e_sb = epool.tile([128, 2, SC], BF16, tag="esb",
                                  name=f"esb{pair}{qc}{t}")
                for i in range(2):
                    p0 = 64 * i
                    nc.tensor.matmul(
                        pse[:, i, 0:w],
                        kt[p0:p0 + DH, 128 * t:128 * (t + 1)],
                        qt[p0:p0 + DH, SC * qc + dd:SC * (qc + 1)],
                        start=True, stop=True,
                    )
                nc.scalar.activation(
                    e_sb[:, :, 0:w], pse[:, :, 0:w],
                    mybir.ActivationFunctionType.Exp,
                    bias=0.0, scale=float(SCALE),
                )
                if diag:
                    # only the first 128 trimmed columns straddle the triangle
                    for i in range(2):
                        nc.vector.tensor_mul(
                            e_sb[:, i, 0:128], e_sb[:, i, 0:128], strip[:, 384:512]
                        )
                return e_sb, dd, w

            def normalize(pair, qc, psos, tail=False):
                # dens for both planes in one partition-0 copy (the fast
                # reciprocal needs base partition 0), then recs, broadcasts,
                # and psum-direct muls; the tail den copy rides the by-then
                # idle scalar engine so it skips the vector backlog
                den = small.tile([1, 2, SC], F32, tag="den", bufs=2,
                                 name=f"den{pair}{qc}")
                if tail:
                    nc.scalar.copy(den[:], psos[64:65, :, :])
                else:
                    nc.vector.tensor_copy(den[:], psos[64:65, :, :])
                recs = []
                for i in range(2):
                    rec = small.tile([1, SC], F32, tag="rec", bufs=4,
                                     name=f"rec{pair}{qc}{i}")
                    nc.vector.reciprocal_approx_fast(rec[:], den[:, i, :])
                    recs.append(rec)
                bcs = []
                for i in range(2):
                    bc = small.tile([64, SC], F32, tag="bc", bufs=2,
                                    name=f"bc{pair}{qc}{i}")
                    nc.gpsimd.partition_broadcast(bc[:], recs[i][:1, :])
                    bcs.append(bc)
                for i in range(2):
                    nc.vector.tensor_mul(
                        at_sb[pair][64 * i:64 * i + DH, SC * qc:SC * (qc + 1)],
                        psos[0:DH, i, :],
                        bcs[i][:],
                    )

            # ---- output projection ----
            # o2 staging groups two s-tiles per DMA ([128, 2, 1024] bf16)
            def out_pair(st0, last=False):
                """Full out-proj (d=0+d=1) for s-tiles st0, st0+1 -> one DMA."""
                o2 = opool.tile([128, 2, E], BF16, tag="o2", name=f"o2{st0}")
                for sti in range(2):
                    st = st0 + sti
                    for jc in range(2):
                        ps = ps_a.tile([128, SC], F32, tag="ps_proj",
                                       name=f"psb3{st}{jc}")
                        for d in range(2):
                            nc.tensor.matmul(
                                ps[:],
                                at_sb[d][:, 128 * st:128 * (st + 1)],
                                wo_sb[:, d, SC * jc:SC * (jc + 1)],
                                start=(d == 0), stop=(d == 1),
                            )
                        eng = nc.scalar if (last and (sti + jc) % 2 == 0) else nc.vector
                        if eng is nc.scalar:
                            eng.copy(o2[:, sti, SC * jc:SC * (jc + 1)], ps[:])
                        else:
                            eng.tensor_copy(o2[:, sti, SC * jc:SC * (jc + 1)], ps[:])
                dma_eng = [nc.sync, nc.gpsimd][(st0 // 2) % 2]
                dma_eng.dma_start(
                    out.ap()[128 * st0:128 * (st0 + 2), :]
                    .rearrange("(a p) j -> p a j", p=128),
                    o2[:],
                )

            # last chunk d-split: pair-0 contributions into bf16 partials
            # (run as fillers inside pair-1's stream), tail adds d=1.
            opart = [pp.tile([128, 2, SC], BF16, tag=f"op{st}", name=f"op{st}")
                     for st in range(4 * (NSC - 1), ST)]

            def d0_partial(st, jc):
                ps = ps_a.tile([128, SC], F32, tag="ps_proj", name=f"psd0{st}{jc}")
                nc.tensor.matmul(
                    ps[:],
                    at_sb[0][:, 128 * st:128 * (st + 1)],
                    wo_sb[:, 0, SC * jc:SC * (jc + 1)],
                    start=True, stop=True,
                )
                nc.vector.tensor_copy(opart[st - 4 * (NSC - 1)][:, jc, :], ps[:])

            def warm_fill(n):
                # dummy matmuls that bridge the final normalize latency so the
                # PE stays at the warm clock for the tail projection
                for j in range(n):
                    ps = ps_a.tile([128, SC], F32, tag="ps_proj", name=f"warm{j}")
                    nc.tensor.matmul(
                        ps[:], at_sb[0][:, 0:128], wo_sb[:, 0, 0:SC],
                        start=True, stop=True, skip_group_check=True,
                    )

            def tail_out(st):
                o1 = opool.tile([128, E], BF16, tag="o1", name=f"o1t{st}")
                ps = ps_e.tile([128, 2, SC], F32, tag="pse", name=f"psd1{st}")
                for jc in range(2):
                    nc.tensor.matmul(
                        ps[:, jc, :],
                        at_sb[1][:, 128 * st:128 * (st + 1)],
                        wo_sb[:, 1, SC * jc:SC * (jc + 1)],
                        start=True, stop=True,
                    )
                # half-tile adds + DMAs so the output drains as it forms
                for jc in range(2):
                    nc.vector.tensor_add(
                        o1[:, SC * jc:SC * (jc + 1)], ps[:, jc, :],
                        opart[st - 4 * (NSC - 1)][:, jc, :],
                    )
                    eng = [nc.sync, nc.scalar, nc.gpsimd][(2 * st + jc) % 3]
                    eng.dma_start(
                        out.ap()[128 * st:128 * (st + 1), SC * jc:SC * (jc + 1)],
                        o1[:, SC * jc:SC * (jc + 1)],
                    )

            # ---- the attention stream with a filler queue ----
            def run_pair(pair, qc, fillers, pre=(), tail=False):
                nt = 4 * (qc + 1)
                nt_eff = max(nt - 3, 1)
                nf = len(fillers)
                popped = 0
                psos = ps_o.tile([128, 2, SC], F32, tag="pso", name=f"pso{pair}{qc}")
                es = {t0: score_tile(pair, qc, t0) for t0 in range(min(2, nt))}
                for f in pre:
                    f()
                # batch two tiles per iteration: scores for tb+2/tb+3, then
                # AVs for tb/tb+1 — halves the 64<->128 row-tiling mode
                # switches on the PE array
                for tb in range(0, nt, 2):
                    for t2 in (tb + 2, tb + 3):
                        if t2 < nt:
                            es[t2] = score_tile(pair, qc, t2)
                    while fillers and popped < (tb + 2) * nf // nt_eff:
                        fillers.popleft()()
                        popped += 1
                    for t in (tb, tb + 1):
                        e_sb, dd, w = es.pop(t)
                        for i in range(2):
                            nc.tensor.matmul(
                                psos[0:DH + 1, i, dd:SC],
                                vst[t][:, 65 * (2 * pair + i):65 * (2 * pair + i) + DH + 1],
                                e_sb[:, i, 0:w],
                                start=(t == 0), stop=(t == nt - 1),
                            )
                normalize(pair, qc, psos, tail=tail)
                while fillers:
                    fillers.popleft()()

            FQ = collections.deque

            # chunk 0: chains first (warms the PE off minimal data), V stores
            # interleaved so scores/exp overlap the xv DMA wait
            q_chain(0, 0)
            k_chain(0, 0)
            run_pair(0, 0, FQ([lambda: vst_chain(1), lambda: vst_chain(2),
                               lambda: vst_chain(3)]),
                     pre=(lambda: q_chain(0, 1), lambda: k_chain(0, 1),
                          lambda: vst_chain(0)))
            run_pair(1, 0, FQ([lambda: q_chain(1, 0), lambda: k_chain(1, 0)]))

            # chunk 1
            f1 = FQ([lambda: q_chain(1, 1), lambda: k_chain(1, 1),
                     lambda: vst_chain(4), lambda: vst_chain(5),
                     lambda: out_pair(0), lambda: vst_chain(6),
                     lambda: vst_chain(7), lambda: out_pair(2)])
            run_pair(0, 1, f1)
            f1b = FQ([lambda: q_chain(2, 0), lambda: k_chain(2, 0),
                      lambda: q_chain(2, 1), lambda: k_chain(2, 1)])
            run_pair(1, 1, f1b)

            # chunk 2
            f2 = FQ([lambda: vst_chain(8), lambda: vst_chain(9),
                     lambda: out_pair(4), lambda: vst_chain(10),
                     lambda: vst_chain(11), lambda: out_pair(6)])
            run_pair(0, 2, f2)
            f2b = FQ([lambda: q_chain(3, 0), lambda: k_chain(3, 0),
                      lambda: q_chain(3, 1), lambda: k_chain(3, 1)])
            run_pair(1, 2, f2b)

            # chunk 3: pair 0 with vst 12-15 + chunk-2 bursts; pair 1 with the
            # d=0 out-proj partials of chunk 3 as fillers
            f3 = FQ([lambda: vst_chain(12), lambda: vst_chain(13),
                     lambda: vst_chain(14), lambda: vst_chain(15),
                     lambda: out_pair(8), lambda: out_pair(10)])
            run_pair(0, 3, f3)
            f3b = FQ([lambda st=st, jc=jc: d0_partial(st, jc)
                      for st in range(12, 16) for jc in range(2)])
            run_pair(1, 3, f3b, tail=True)

            # tail: warm bridges over the last normalize and between the
            # DVE-paced tile evacuations, d=1 matmuls + combine + DMA
            warm_fill(10)
            for st in range(12, 16):
                tail_out(st)
                if st < 15:
                    warm_fill(3)

    nc.compile()
    return nc


def _get_nc():
    if "nc" not in _CACHED:
        _CACHED["nc"] = _build()
    return _CACHED["nc"]


def _in_maps(q, k, v, Wq, bq, Wk, bk, Wv, bv, Wo, bo):
    B = q.shape[0]
    f32 = np.float32
    xT = {}
    for b in range(B):
        xT[("q", b)] = np.ascontiguousarray(q[b].T).astype(NP_BF16)
        xT[("k", b)] = np.ascontiguousarray(k[b].T).astype(NP_BF16)
        xT[("v", b)] = np.ascontiguousarray(v[b].T).astype(NP_BF16)
    maps = []
    for c in range(8):
        b, g = c // 4, c % 4
        rows = slice(DG * g, DG * (g + 1))
        maps.append({
            "xqT": xT[("q", b)],
            "xkT": xT[("k", b)],
            "xvT": xT[("v", b)],
            "wqT": np.ascontiguousarray(Wq[rows, :].T).astype(NP_BF16),
            "wkT": np.ascontiguousarray(Wk[rows, :].T).astype(NP_BF16),
            "wvT": np.ascontiguousarray(Wv[rows, :].T).astype(NP_BF16),
            "woT": np.ascontiguousarray(Wo[:, rows].T).astype(NP_BF16),
            "bq": np.ascontiguousarray(bq[rows], dtype=f32),
            "bk": np.ascontiguousarray(bk[rows], dtype=f32),
            "bv": np.ascontiguousarray(bv[rows], dtype=f32),
        })
    return maps


def _run(inputs, trace=False):
    nc = _get_nc()
    maps = _in_maps(
        inputs["q"], inputs["k"], inputs["v"],
        inputs["Wq"], inputs["bq"], inputs["Wk"], inputs["bk"],
        inputs["Wv"], inputs["bv"], inputs["Wo"], inputs["bo"],
    )
    res = run_bass_kernel_spmd(nc, maps, list(range(8)), trace=trace)
    parts = [np.asarray(r["out"], dtype=np.float32) for r in res.results]
    bo_row = np.asarray(inputs["bo"], dtype=np.float32)
    out = np.stack([
        parts[0] + parts[1] + parts[2] + parts[3] + bo_row,
        parts[4] + parts[5] + parts[6] + parts[7] + bo_row,
    ]).astype(np.float32)
    return out, res


def kernel(**inputs):
    out, _ = _run(inputs, trace=False)
    return out


# revision 24
# speedup vs baseline: 2.9057x; 2.9057x over previous
"""Multi-head attention (B=2, S=2048, D=1024, H=16, causal) on 8 trn2 cores.

Sharding: core c -> batch b = c//4, head group g = c%4 (4 heads of 64 dims):
data parallel over batch, tensor/head parallel within it (W_q/W_k/W_v split
column-wise, W_o row-wise per head group).  Each core computes Q/K/V
projections for its head group over the full sequence, causal flash-style
attention, and the partial output projection A_g @ Wo.T[g_rows, :].  The host
pre-transposes activations/weight slices to fp16, sums the 4 bf16 output
partials per batch (the row-parallel unshard), and adds bo.

v2 schedule (vs the phase-ordered baseline):
  - DMA loads stream in compute priority order (wq/wk + x chunk-0 columns
    first, per-e granularity for the first chunk so the first projection
    chain starts ~3us in), V/x data for later chunks trickle behind.
  - V-store chains, next-chunk Q/K chains and previous-chunk output bursts
    are a single filler queue drained one item per attention tile, so the
    PE never waits on the ACT exp stream.
  - exp is issued per head plane ([128,w] x2 instead of [128,2,w]) so the
    AV matmul of plane 0 starts while plane 1 is still exping.
  - softmax normalization reads PSUM directly (reciprocal + multiply), no
    den/A-copy staging; causal strip muls run on gpsimd.
  - output partials are bf16, copied into [128,2,1024] staging tiles and
    DMA'd two s-tiles at a time.
  - last chunk: pair-0 out-proj contributions (d=0 matmuls) run as fillers
    inside pair-1's attention stream into bf16 partials; the tail only runs
    the 8 d=1 matmuls + add-combine + DMA.
"""

import collections

import ml_dtypes
import numpy as np

import concourse.bacc as bacc
import concourse.mybir as mybir
import concourse.tile as tile
from concourse.bass_utils import run_bass_kernel_spmd

F32 = mybir.dt.float32
BF16 = mybir.dt.float16  # fp16: same PE speed as bf16, 4x the mantissa
F8 = mybir.dt.float8e4
NP_BF16 = np.float16
NP_F8 = ml_dtypes.float8_e4m3
DR = mybir.MatmulPerfMode.DoubleRow

S = 2048        # sequence length
E = 1024        # model dim (contraction for projections)
DG = 256        # head-group dim (4 heads x 64)
DH = 64         # head dim
NH = 4          # heads per core
ET = E // 128   # 8 e-tiles
ST = S // 128   # 16 s-tiles
SC = 512        # sequence chunk (psum free dim)
NSC = S // SC   # 4 chunks
SCALE = 1.0 / np.sqrt(DH)

_CACHED = {}


def _build():
    nc = bacc.Bacc("TRN2", target_bir_lowering=False, debug=False, num_devices=8)

    xqT = nc.dram_tensor("xqT", [E, S], BF16, kind="ExternalInput")
    xkT = nc.dram_tensor("xkT", [E, S], BF16, kind="ExternalInput")
    xvT = nc.dram_tensor("xvT", [E, S], BF16, kind="ExternalInput")
    wqT = nc.dram_tensor("wqT", [E, DG], BF16, kind="ExternalInput")
    wkT = nc.dram_tensor("wkT", [E, DG], BF16, kind="ExternalInput")
    wvT = nc.dram_tensor("wvT", [E, DG], BF16, kind="ExternalInput")
    woT = nc.dram_tensor("woT", [DG, E], BF16, kind="ExternalInput")
    bq = nc.dram_tensor("bq", [DG], F32, kind="ExternalInput")
    bk = nc.dram_tensor("bk", [DG], F32, kind="ExternalInput")
    bv = nc.dram_tensor("bv", [DG], F32, kind="ExternalInput")
    out = nc.dram_tensor("out", [S, E], BF16, kind="ExternalOutput")

    with tile.TileContext(nc) as tc:
        with (
            tc.tile_pool(name="persist", bufs=1) as pp,
            tc.tile_pool(name="xin", bufs=1) as xin,
            tc.tile_pool(name="epool", bufs=6) as epool,
            tc.tile_pool(name="opool", bufs=2) as opool,
            tc.tile_pool(name="small", bufs=2) as small,
            tc.tile_pool(name="ps_a", bufs=2, space="PSUM") as ps_a,
            tc.tile_pool(name="ps_e", bufs=2, space="PSUM") as ps_e,
            tc.tile_pool(name="ps_o", bufs=1, space="PSUM") as ps_o,
        ):
            # ---- persistent tiles ----
            wq_sb = pp.tile([128, ET, DG], BF16, tag="wq")
            wk_sb = pp.tile([128, ET, DG], BF16, tag="wk")
            wv_sb = pp.tile([128, ET, DG], BF16, tag="wv")
            wo_sb = pp.tile([128, 2, E], BF16, tag="wo")
            bq_sb = pp.tile([128, 2], F32, tag="bq")
            bk_sb = pp.tile([128, 2], F32, tag="bk")
            bv_sb = pp.tile([1, DG], F32, tag="bv")
            bvb = pp.tile([128, DG], F32, tag="bvb")

            xq_sb = xin.tile([128, ET, S], BF16, tag="xq", name="xq")
            xk_sb = xin.tile([128, ET, S], BF16, tag="xk", name="xk")
            xv_sb = xin.tile([128, ET, S], BF16, tag="xv", name="xv")

            # ---- DMA stream, priority ordered ----
            # band 0: projection weights + chunk-0 activations (per-e so the
            # first chain starts as soon as its first slices land)
            nc.gpsimd.dma_start(bq_sb[:], bq.ap().rearrange("(a p) -> p a", p=128))
            nc.gpsimd.dma_start(bk_sb[:], bk.ap().rearrange("(a p) -> p a", p=128))
            nc.gpsimd.dma_start(bv_sb[:], bv.ap().rearrange("(o d) -> o d", o=1))
            wq_view = wqT.ap().rearrange("(a p) d -> p a d", p=128)
            nc.gpsimd.dma_start(wq_sb[:, 0:ET // 2, :], wq_view[:, 0:ET // 2, :])
            nc.gpsimd.dma_start(wq_sb[:, ET // 2:, :], wq_view[:, ET // 2:, :])
            nc.gpsimd.dma_start(wk_sb[:], wkT.ap().rearrange("(a p) d -> p a d", p=128))
            for e in range(ET):
                nc.sync.dma_start(xq_sb[:, e, 0:SC],
                                  xqT.ap()[128 * e:128 * (e + 1), 0:SC])
                nc.scalar.dma_start(xk_sb[:, e, 0:SC],
                                    xkT.ap()[128 * e:128 * (e + 1), 0:SC])
            nc.gpsimd.dma_start(wv_sb[:], wvT.ap().rearrange("(a p) d -> p a d", p=128))
            # band 1: V chunk 0, then later x chunks in need order
            xv_view = xvT.ap().rearrange("(a p) s -> p a s", p=128)
            xq_view = xqT.ap().rearrange("(a p) s -> p a s", p=128)
            xk_view = xkT.ap().rearrange("(a p) s -> p a s", p=128)
            nc.sync.dma_start(xv_sb[:, :, 0:SC], xv_view[:, :, 0:SC])
            nc.scalar.dma_start(xq_sb[:, :, SC:2 * SC], xq_view[:, :, SC:2 * SC])
            nc.sync.dma_start(xk_sb[:, :, SC:2 * SC], xk_view[:, :, SC:2 * SC])
            nc.gpsimd.dma_start(wo_sb[:], woT.ap().rearrange("(a p) j -> p a j", p=128))
            nc.scalar.dma_start(xv_sb[:, :, SC:2 * SC], xv_view[:, :, SC:2 * SC])
            nc.sync.dma_start(xv_sb[:, :, 2 * SC:3 * SC], xv_view[:, :, 2 * SC:3 * SC])
            nc.scalar.dma_start(xq_sb[:, :, 2 * SC:4 * SC], xq_view[:, :, 2 * SC:4 * SC])
            nc.sync.dma_start(xk_sb[:, :, 2 * SC:4 * SC], xk_view[:, :, 2 * SC:4 * SC])
            nc.scalar.dma_start(xv_sb[:, :, 3 * SC:4 * SC], xv_view[:, :, 3 * SC:4 * SC])

            # bias row broadcast for the V-store add (gpsimd, once)
            nc.gpsimd.partition_broadcast(bvb[:], bv_sb[:1, :])

            # causal strip: strip[p, f] = 1.0 if f - p >= 384 else 0.0
            strip = pp.tile([128, 896], BF16, tag="strip")
            nc.gpsimd.memset(strip[:], 1.0)
            nc.gpsimd.affine_select(
                out=strip[:],
                in_=strip[:],
                compare_op=mybir.AluOpType.is_ge,
                fill=0.0,
                base=-384,
                pattern=[[1, 896]],
                channel_multiplier=-1,
            )

            # ---- compute helpers ----
            qt_sb = [pp.tile([128, S], BF16, tag=f"qt{d}", name=f"qt{d}") for d in range(2)]
            kt_sb = [pp.tile([128, S], BF16, tag=f"kt{d}", name=f"kt{d}") for d in range(2)]
            at_sb = [pp.tile([128, S], BF16, tag=f"at{d}", name=f"at{d}") for d in range(2)]
            vst = [pp.tile([128, NH * (DH + 1)], BF16, tag=f"vst{st}", name=f"vst{st}")
                   for st in range(ST)]
            for st in range(ST):
                nc.gpsimd.memset(vst[st][:], 1.0)

            def proj_chain(x_sb, w_sb, b_sb, dst, sc, d):
                ps = ps_a.tile([128, SC], F32, tag="ps_proj",
                               name=f"pj{dst[0].name}{sc}{d}")
                for e in range(ET):
                    nc.tensor.matmul(
                        ps[:],
                        w_sb[:, e, 128 * d:128 * (d + 1)],
                        x_sb[:, e, SC * sc:SC * (sc + 1)],
                        start=(e == 0),
                        stop=(e == ET - 1),
                    )
                nc.vector.tensor_scalar_add(
                    dst[d][:, SC * sc:SC * (sc + 1)], ps[:], b_sb[:, d:d + 1]
                )

            def q_chain(sc, d):
                proj_chain(xq_sb, wq_sb, bq_sb, qt_sb, sc, d)

            def k_chain(sc, d):
                proj_chain(xk_sb, wk_sb, bk_sb, kt_sb, sc, d)

            def vst_chain(st):
                ps = ps_a.tile([128, SC], F32, tag="ps_proj", name=f"psv{st}")
                col = 128 * st
                for e in range(ET):
                    nc.tensor.matmul(
                        ps[:, 0:DG],
                        xv_sb[:, e, col:col + 128],
                        wv_sb[:, e, :],
                        start=(e == 0),
                        stop=(e == ET - 1),
                    )
                # psum + broadcast bias -> v columns of the store (ones cols stay)
                nc.vector.tensor_add(
                    vst[st][:].rearrange("p (h x) -> p h x", h=NH)[:, :, 0:DH],
                    ps[:, 0:DG].rearrange("p (h x) -> p h x", h=NH),
                    bvb[:].rearrange("p (h x) -> p h x", h=NH),
                )

            def score_tile(pair, qc, t):
                """Both heads' score matmuls -> one 2-bank psum; per-plane exp."""
                qt, kt = qt_sb[pair], kt_sb[pair]
                diag = t >= 4 * qc
                dd = 128 * t - SC * qc if diag else 0
                w = SC - dd
                pse = ps_e.tile([128, 2, SC], F32, tag="pse", name=f"pse{pair}{qc}{t}")
                e_sb = epool.tile([128, 2, SC], BF16, tag="esb",
                                  name=f"esb{pair}{qc}{t}")
                for i in range(2):
                    p0 = 64 * i
                    nc.tensor.matmul(
                        pse[:, i, 0:w],
                        kt[p0:p0 + DH, 128 * t:128 * (t + 1)],
                        qt[p0:p0 + DH, SC * qc + dd:SC * (qc + 1)],
                        start=True, stop=True,
                    )
                nc.scalar.activation(
                    e_sb[:, :, 0:w], pse[:, :, 0:w],
                    mybir.ActivationFunctionType.Exp,
                    bias=0.0, scale=float(SCALE),
                )
                if diag:
                    # only the first 128 trimmed columns straddle the triangle
                    for i in range(2):
                        nc.vector.tensor_mul(
                            e_sb[:, i, 0:128], e_sb[:, i, 0:128], strip[:, 384:512]
                        )
                return e_sb, dd, w

            def normalize(pair, qc, psos, tail=False):
                # dens for both planes in one partition-0 copy (the fast
                # reciprocal needs base partition 0), then recs, broadcasts,
                # and psum-direct muls; the tail den copy rides the by-then
                # idle scalar engine so it skips the vector backlog
                den = small.tile([1, 2, SC], F32, tag="den", bufs=2,
                                 name=f"den{pair}{qc}")
                if tail:
                    nc.scalar.copy(den[:], psos[64:65, :, :])
                else:
                    nc.vector.tensor_copy(den[:], psos[64:65, :, :])
                recs = []
                for i in range(2):
                    rec = small.tile([1, SC], F32, tag="rec", bufs=4,
                                     name=f"rec{pair}{qc}{i}")
                    nc.vector.reciprocal_approx_fast(rec[:], den[:, i, :])
                    recs.append(rec)
                bcs = []
                for i in range(2):
                    bc = small.tile([64, SC], F32, tag="bc", bufs=2,
                                    name=f"bc{pair}{qc}{i}")
                    nc.gpsimd.partition_broadcast(bc[:], recs[i][:1, :])
                    bcs.append(bc)
                for i in range(2):
                    nc.vector.tensor_mul(
                        at_sb[pair][64 * i:64 * i + DH, SC * qc:SC * (qc + 1)],
                        psos[0:DH, i, :],
                        bcs[i][:],
                    )

            # ---- output projection ----
            # o2 staging groups two s-tiles per DMA ([128, 2, 1024] bf16)
            def out_pair(st0, last=False):
                """Full out-proj (d=0+d=1) for s-tiles st0, st0+1 -> one DMA."""
                o2 = opool.tile([128, 2, E], BF16, tag="o2", name=f"o2{st0}")
                for sti in range(2):
                    st = st0 + sti
                    for jc in range(2):
                        ps = ps_a.tile([128, SC], F32, tag="ps_proj",
                                       name=f"psb3{st}{jc}")
                        for d in range(2):
                            nc.tensor.matmul(
                                ps[:],
                                at_sb[d][:, 128 * st:128 * (st + 1)],
                                wo_sb[:, d, SC * jc:SC * (jc + 1)],
                                start=(d == 0), stop=(d == 1),
                            )
                        eng = nc.scalar if (last and (sti + jc) % 2 == 0) else nc.vector
                        if eng is nc.scalar:
                            eng.copy(o2[:, sti, SC * jc:SC * (jc + 1)], ps[:])
                        else:
                            eng.tensor_copy(o2[:, sti, SC * jc:SC * (jc + 1)], ps[:])
                dma_eng = [nc.sync, nc.gpsimd][(st0 // 2) % 2]
                dma_eng.dma_start(
                    out.ap()[128 * st0:128 * (st0 + 2), :]
                    .rearrange("(a p) j -> p a j", p=128),
                    o2[:],
                )

            # last chunk d-split: pair-0 contributions into bf16 partials
            # (run as fillers inside pair-1's stream), tail adds d=1.
            opart = [pp.tile([128, 2, SC], BF16, tag=f"op{st}", name=f"op{st}")
                     for st in range(4 * (NSC - 1), ST)]

            def d0_partial(st, jc):
                ps = ps_a.tile([128, SC], F32, tag="ps_proj", name=f"psd0{st}{jc}")
                nc.tensor.matmul(
                    ps[:],
                    at_sb[0][:, 128 * st:128 * (st + 1)],
                    wo_sb[:, 0, SC * jc:SC * (jc + 1)],
                    start=True, stop=True,
                )
                nc.vector.tensor_copy(opart[st - 4 * (NSC - 1)][:, jc, :], ps[:])

            def warm_fill(n):
                # dummy matmuls that bridge the final normalize latency so the
                # PE stays at the warm clock for the tail projection
                for j in range(n):
                    ps = ps_a.tile([128, SC], F32, tag="ps_proj", name=f"warm{j}")
                    nc.tensor.matmul(
                        ps[:], at_sb[0][:, 0:128], wo_sb[:, 0, 0:SC],
                        start=True, stop=True, skip_group_check=True,
                    )

            def tail_out(st):
                o1 = opool.tile([128, E], BF16, tag="o1", name=f"o1t{st}")
                ps = ps_e.tile([128, 2, SC], F32, tag="pse", name=f"psd1{st}")
                for jc in range(2):
                    nc.tensor.matmul(
                        ps[:, jc, :],
                        at_sb[1][:, 128 * st:128 * (st + 1)],
                        wo_sb[:, 1, SC * jc:SC * (jc + 1)],
                        start=True, stop=True,
                    )
                # half-tile adds + DMAs so the output drains as it forms
                for jc in range(2):
                    nc.vector.tensor_add(
                        o1[:, SC * jc:SC * (jc + 1)], ps[:, jc, :],
                        opart[st - 4 * (NSC - 1)][:, jc, :],
                    )
                    eng = [nc.sync, nc.scalar, nc.gpsimd][(2 * st + jc) % 3]
                    eng.dma_start(
                        out.ap()[128 * st:128 * (st + 1), SC * jc:SC * (jc + 1)],
                        o1[:, SC * jc:SC * (jc + 1)],
                    )

            # ---- the attention stream with a filler queue ----
            def run_pair(pair, qc, fillers, pre=(), tail=False):
                nt = 4 * (qc + 1)
                nt_eff = max(nt - 3, 1)
                nf = len(fillers)
                popped = 0
                psos = ps_o.tile([128, 2, SC], F32, tag="pso", name=f"pso{pair}{qc}")
                es = {t0: score_tile(pair, qc, t0) for t0 in range(min(2, nt))}
                for f in pre:
                    f()
                for t in range(nt):
                    if t + 2 < nt:
                        es[t + 2] = score_tile(pair, qc, t + 2)
                    # spread fillers evenly, finishing a few slots early
                    while fillers and popped < (t + 1) * nf // nt_eff:
                        fillers.popleft()()
                        popped += 1
                    e_sb, dd, w = es.pop(t)
                    for i in range(2):
                        nc.tensor.matmul(
                            psos[0:DH + 1, i, dd:SC],
                            vst[t][:, 65 * (2 * pair + i):65 * (2 * pair + i) + DH + 1],
                            e_sb[:, i, 0:w],
                            start=(t == 0), stop=(t == nt - 1),
                        )
                normalize(pair, qc, psos, tail=tail)
                while fillers:
                    fillers.popleft()()

            FQ = collections.deque

            # chunk 0: chains first (warms the PE off minimal data), V stores
            # interleaved so scores/exp overlap the xv DMA wait
            q_chain(0, 0)
            k_chain(0, 0)
            run_pair(0, 0, FQ([lambda: vst_chain(1), lambda: vst_chain(2),
                               lambda: vst_chain(3)]),
                     pre=(lambda: q_chain(0, 1), lambda: k_chain(0, 1),
                          lambda: vst_chain(0)))
            run_pair(1, 0, FQ([lambda: q_chain(1, 0), lambda: k_chain(1, 0)]))

            # chunk 1
            f1 = FQ([lambda: q_chain(1, 1), lambda: k_chain(1, 1),
                     lambda: vst_chain(4), lambda: vst_chain(5),
                     lambda: out_pair(0), lambda: vst_chain(6),
                     lambda: vst_chain(7), lambda: out_pair(2)])
            run_pair(0, 1, f1)
            f1b = FQ([lambda: q_chain(2, 0), lambda: k_chain(2, 0),
                      lambda: q_chain(2, 1), lambda: k_chain(2, 1)])
            run_pair(1, 1, f1b)

            # chunk 2
            f2 = FQ([lambda: vst_chain(8), lambda: vst_chain(9),
                     lambda: out_pair(4), lambda: vst_chain(10),
                     lambda: vst_chain(11), lambda: out_pair(6)])
            run_pair(0, 2, f2)
            f2b = FQ([lambda: q_chain(3, 0), lambda: k_chain(3, 0),
                      lambda: q_chain(3, 1), lambda: k_chain(3, 1)])
            run_pair(1, 2, f2b)

            # chunk 3: pair 0 with vst 12-15 + chunk-2 bursts; pair 1 with the
            # d=0 out-proj partials of chunk 3 as fillers
            f3 = FQ([lambda: vst_chain(12), lambda: vst_chain(13),
                     lambda: vst_chain(14), lambda: vst_chain(15),
                     lambda: out_pair(8), lambda: out_pair(10)])
            run_pair(0, 3, f3)
            f3b = FQ([lambda st=st, jc=jc: d0_partial(st, jc)
                      for st in range(12, 16) for jc in range(2)])
            run_pair(1, 3, f3b, tail=True)

            # tail: warm bridges over the last normalize and between the
            # DVE-paced tile evacuations, d=1 matmuls + combine + DMA
            warm_fill(10)
            for st in range(12, 16):
                tail_out(st)
                if st < 15:
                    warm_fill(3)

    nc.compile()
    return nc


def _get_nc():
    if "nc" not in _CACHED:
        _CACHED["nc"] = _build()
    return _CACHED["nc"]


def _in_maps(q, k, v, Wq, bq, Wk, bk, Wv, bv, Wo, bo):
    B = q.shape[0]
    f32 = np.float32
    xT = {}
    for b in range(B):
        xT[("q", b)] = np.ascontiguousarray(q[b].T).astype(NP_BF16)
        xT[("k", b)] = np.ascontiguousarray(k[b].T).astype(NP_BF16)
        xT[("v", b)] = np.ascontiguousarray(v[b].T).astype(NP_BF16)
    maps = []
    for c in range(8):
        b, g = c // 4, c % 4
        rows = slice(DG * g, DG * (g + 1))
        maps.append({
            "xqT": xT[("q", b)],
            "xkT": xT[("k", b)],
            "xvT": xT[("v", b)],
            "wqT": np.ascontiguousarray(Wq[rows, :].T).astype(NP_BF16),
            "wkT": np.ascontiguousarray(Wk[rows, :].T).astype(NP_BF16),
            "wvT": np.ascontiguousarray(Wv[rows, :].T).astype(NP_BF16),
            "woT": np.ascontiguousarray(Wo[:, rows].T).astype(NP_BF16),
            "bq": np.ascontiguousarray(bq[rows], dtype=f32),
            "bk": np.ascontiguousarray(bk[rows], dtype=f32),
            "bv": np.ascontiguousarray(bv[rows], dtype=f32),
        })
    return maps


def _run(inputs, trace=False):
    nc = _get_nc()
    maps = _in_maps(
        inputs["q"], inputs["k"], inputs["v"],
        inputs["Wq"], inputs["bq"], inputs["Wk"], inputs["bk"],
        inputs["Wv"], inputs["bv"], inputs["Wo"], inputs["bo"],
    )
    res = run_bass_kernel_spmd(nc, maps, list(range(8)), trace=trace)
    parts = [np.asarray(r["out"], dtype=np.float32) for r in res.results]
    bo_row = np.asarray(inputs["bo"], dtype=np.float32)
    out = np.stack([
        parts[0] + parts[1] + parts[2] + parts[3] + bo_row,
        parts[4] + parts[5] + parts[6] + parts[7] + bo_row,
    ]).astype(np.float32)
    return out, res


def kernel(**inputs):
    out, _ = _run(inputs, trace=False)
    return out
